# revision 6
# baseline (speedup 1.0000x reference)
"""Bass/Trainium2 kernel for nn_LocalAggregator (GNN message passing).

Math per batch b (hidden [64,128], adj [64,64] in {0..4}, a [4,128]):
    e_k[i,j] = leakyrelu_{0.2}( sum_d hidden[i,d]*hidden[j,d]*a[k,d] )
    alpha    = softmax_j( where(adj==k+1, e_k, -9e15) )
    out      = alpha @ hidden

Device strategy (8 cores, pure batch data-parallel, 64 batches/core).
Batches are fused in PAIRS (2 batches = 128 nodes -> full-width matmuls;
cross-batch terms are computed but killed by the adjacency mask), and
processed in OCTs (4 pairs = 8 batches) so element-wise ops run on
[128, 2048] tiles that amortize per-op overheads.

Per oct q (tiles: hT [d, (pair,i)], hh [j2b, (pair, d+ones)],
adjT [j2b, (pair, i)] block-diagonal-transposed adjacency):
  - w_all[d,(pair,k,i)] = hT * a_k           (4x tensor_scalar, DVE 4x mode)
  - e2[j2b,(k,i)] = hT_pair^T @ w_all_pair   (1 matmul per pair, PSUM f32)
    e_k is symmetric, so this tile read as [j,(k,i)] is e_k[i,j].
  - lr = Prelu(e2) evacuates PSUM on ACT
  - A[j,(pair,k,i)] = (adjT != k+1) * -40    (4x dual-op tensor_scalar)
  - esel = max_k (lr + A): selected value where adj==k+1, else <= -35
    (leakyrelu commutes with one-hot selection; exp(-35) underflows to
    exactly 0 in fp16, which also kills the cross-batch block entries)
  - w = Exp(esel) on ACT ([128,512] only, 4x smaller than lr)
  - out_pair[i,d] = sum_j w[j,i]*hh[j,d]; ones-col gives denominator s_i
  - unnormalized out + s shipped fp16; host divides and casts.
"""

import numpy as np
import ml_dtypes

from contextlib import ExitStack

import concourse.bass as bass
import concourse.tile as tile
from concourse import bacc, mybir
from concourse._compat import with_exitstack
from concourse.bass_utils import run_bass_kernel_spmd

F16 = mybir.dt.float16
F32 = mybir.dt.float32
ALU = mybir.AluOpType
ACTF = mybir.ActivationFunctionType

B, N, D, K = 512, 64, 128, 4
NCORES = 8
BPC = B // NCORES          # 64 batches per core
NOCT = BPC // 8            # 8 octs of 8 batches (4 pairs) per core
HHW = 132                  # hidden cols + ones col + pad (128 data, 1 ones, 3 zero)
CIN = 512 + 4 * HHW + 512  # blob cols: hT(512) | hh(528) | adjT(512)
MASKV = -40.0              # additive mask; exp(-35) underflows fp16 -> 0


@with_exitstack
def _kernel_body(ctx, tc, blob_d, aT_d, out_d):
    nc = tc.nc

    const_pool = ctx.enter_context(tc.tile_pool(name="const", bufs=1))
    in_pool = ctx.enter_context(tc.tile_pool(name="inp", bufs=3))
    work_pool = ctx.enter_context(tc.tile_pool(name="work", bufs=3))
    psum_pool = ctx.enter_context(tc.tile_pool(name="psum", bufs=2, space="PSUM"))
    opsum_pool = ctx.enter_context(tc.tile_pool(name="opsum", bufs=2, space="PSUM"))
    out_pool = ctx.enter_context(tc.tile_pool(name="outp", bufs=3))

    a_sb = const_pool.tile([128, 4], F32)          # a^T : [d, k]
    nc.sync.dma_start(out=a_sb[:], in_=aT_d[:, :])

    for q in range(NOCT):
        blob = in_pool.tile([128, CIN], F16, tag="blob")
        nc.sync.dma_start(out=blob[:], in_=blob_d[q])
        hT = blob[:, 0:512]                       # [d, (pair, i)]
        hh = blob[:, 512 : 512 + 4 * HHW]         # [j2b, (pair, d+ones)]
        adjT = blob[:, 512 + 4 * HHW : CIN]       # [j2b, (pair, i)]

        # ---- w_all[d, (k, pair, i)] = hT * a_k (k-major: contiguous
        #      tensor_scalar outputs keep the DVE in 4x mode) ----
        w_all = work_pool.tile([128, 2048], F16, tag="w_all")
        for k in range(K):
            nc.vector.tensor_scalar(
                w_all[:, k * 512 : (k + 1) * 512], hT,
                a_sb[:, k : k + 1], None, ALU.mult)

        # ---- A[j, (k, pair, i)] = (adjT != k+1) * MASKV ----
        amask = work_pool.tile([128, 2048], F16, tag="amask")
        for k in range(K):
            nc.vector.tensor_scalar(
                amask[:, k * 512 : (k + 1) * 512], adjT,
                float(k + 1), MASKV, ALU.not_equal, ALU.mult)

        # ---- e2 per pair (PSUM f32), Prelu-evacuated to lr (fp16) ----
        # rhs is a strided view: pair p's (k, i) columns of k-major w_all.
        wv = w_all[:].rearrange("p (k a i) -> p a k i", k=4, a=4)
        lr = work_pool.tile([128, 2048], F16, tag="lr")
        for duo in range(2):
            e2 = psum_pool.tile([128, 1024], F32, tag="e2")
            for pp in range(2):
                p = 2 * duo + pp
                nc.tensor.matmul(
                    e2[:, pp * 512 : (pp + 1) * 512],
                    lhsT=hT[:, p * 128 : (p + 1) * 128],
                    rhs=wv[:, p, :, :],
                    start=True, stop=True,
                )
            nc.scalar.activation(
                lr[:, duo * 1024 : (duo + 1) * 1024], e2[:],
                ACTF.Prelu, alpha=0.2)

        # ---- esel[j, (pair, i)] = max_k (lr + A) ----
        # lr columns are (pair, k, i); amask's are (k, pair, i).
        z = work_pool.tile([128, 2048], F16, tag="z")
        zpv = z[:].rearrange("p (a k i) -> p a k i", a=4, k=4)
        lrv = lr[:].rearrange("p (a k i) -> p a k i", a=4, k=4)
        amv = amask[:].rearrange("p (k a i) -> p a k i", k=4, a=4)
        nc.vector.tensor_tensor(zpv, lrv, amv, ALU.add)
        zv = z[:].rearrange("p (a k i) -> p a k i", a=4, k=4)
        t2 = work_pool.tile([128, 1024], F16, tag="t2")
        t2v = t2[:].rearrange("p (a k i) -> p a k i", a=4, k=2)
        nc.vector.tensor_tensor(t2v, zv[:, :, 0:2, :], zv[:, :, 2:4, :], ALU.max)
        esel = work_pool.tile([128, 512], F16, tag="esel")
        eselv = esel[:].rearrange("p (a i) -> p a i", a=4)
        nc.vector.tensor_tensor(eselv, t2v[:, :, 0, :], t2v[:, :, 1, :], ALU.max)

        # ---- w[j, (pair, i)] = exp(esel) : masked entries -> exactly 0 ----
        w = work_pool.tile([128, 512], F16, tag="w")
        nc.scalar.activation(w[:], esel[:], ACTF.Exp)

        # ---- out_pair[i, 0:128] = sum_j w[j,i] h[j,d]; col 128 = denom ----
        osum = opsum_pool.tile([128, 1024], F32, tag="osum")
        for p in range(4):
            nc.tensor.matmul(
                osum[:, p * 256 : p * 256 + HHW],
                lhsT=w[:, p * 128 : (p + 1) * 128],
                rhs=hh[:, p * HHW : (p + 1) * HHW],
                start=True, stop=True,
            )

        # ---- evacuate (unnormalized) to fp16 and store ----
        # alternate the evacuation engine to balance ACT/DVE load
        osb = out_pool.tile([128, 4 * HHW], F16, tag="osb")
        osbv = osb[:].rearrange("p (a c) -> p a c", a=4)
        osumv = osum[:].rearrange("p (a c) -> p a c", a=4)[:, :, 0:HHW]
        if q % 2 == 0:
            nc.scalar.activation(osbv, osumv, ACTF.Copy)
        else:
            nc.vector.tensor_scalar(osbv, osumv, 1.0, None, ALU.mult)
        nc.sync.dma_start(out=out_d[q], in_=osb[:])


def build_nc():
    nc = bacc.Bacc("TRN2", target_bir_lowering=False, debug=False)
    blob_d = nc.dram_tensor("blob", [NOCT, 128, CIN], F16, kind="ExternalInput").ap()
    aT_d = nc.dram_tensor("at", [128, 4], F32, kind="ExternalInput").ap()
    out_d = nc.dram_tensor("out", [NOCT, 128, 4 * HHW], F16,
                           kind="ExternalOutput").ap()
    with tile.TileContext(nc) as tc:
        _kernel_body(tc, blob_d, aT_d, out_d)
    nc.compile()
    return nc


def prep_inputs(hidden, adj, a):
    """Host-side packing: fp16 casts, pair-fused block layouts, shards."""
    hidden = np.asarray(hidden, dtype=np.float32)
    adj = np.asarray(adj)
    a = np.asarray(a, dtype=np.float32)

    h16 = hidden.astype(np.float16)                          # [B, 64, 128]

    # hT[b-pairs]: [d, (pair, v)] with v = (u*64 + i), batch = 2*pair_g + u
    hT = (h16.transpose(0, 2, 1)                             # [B, d, i]
          .reshape(B // 2, 2, D, N)                          # [pg, u, d, i]
          .transpose(0, 2, 1, 3)                             # [pg, d, u, i]
          .reshape(B // 2, D, 2 * N))                        # [pg, d, v]

    # hh[pg, v, c]: row v = h[2pg + v//64, v%64, :] plus ones col
    hh = np.zeros((B // 2, 2 * N, HHW), dtype=np.float16)
    hh[:, :, 0:D] = h16.reshape(B // 2, 2 * N, D)
    hh[:, :, D] = np.float16(1.0)

    # adjT block tile [pg, x, y] = adj[2pg + x//64, y%64, x%64] if same half
    adjT = np.zeros((B // 2, 2 * N, 2 * N), dtype=np.float16)
    at = adj.transpose(0, 2, 1).astype(np.float16)           # at[b, j, i]
    adjT[:, 0:N, 0:N] = at[0::2]
    adjT[:, N:2 * N, N:2 * N] = at[1::2]

    aT = np.ascontiguousarray(a.T).astype(np.float32)        # [128, 4]

    # blob[oct, 128, CIN] per core: hT(4 pairs) | hh(4 pairs) | adjT(4 pairs)
    PPC = BPC // 2                                           # 32 pairs per core
    in_maps = []
    for c in range(NCORES):
        psl = slice(c * PPC, (c + 1) * PPC)
        hT_c = hT[psl].reshape(NOCT, 4, D, 2 * N)            # [q, pair, d, v]
        hh_c = hh[psl].reshape(NOCT, 4, 2 * N, HHW)
        adjT_c = adjT[psl].reshape(NOCT, 4, 2 * N, 2 * N)
        blob = np.empty((NOCT, 128, CIN), dtype=np.float16)
        blob[:, :, 0:512] = hT_c.transpose(0, 2, 1, 3).reshape(NOCT, 128, 512)
        blob[:, :, 512:512 + 4 * HHW] = (
            hh_c.transpose(0, 2, 1, 3).reshape(NOCT, 128, 4 * HHW))
        blob[:, :, 512 + 4 * HHW:CIN] = (
            adjT_c.transpose(0, 2, 1, 3).reshape(NOCT, 128, 512))
        in_maps.append({"blob": np.ascontiguousarray(blob), "at": aT})
    return in_maps


_NC_CACHE = {}


def run_device(hidden, adj, a, **spmd_kwargs):
    if "nc" not in _NC_CACHE:
        _NC_CACHE["nc"] = build_nc()
    nc = _NC_CACHE["nc"]
    in_maps = prep_inputs(hidden, adj, a)
    res = run_bass_kernel_spmd(nc, in_maps, list(range(NCORES)), **spmd_kwargs)
    # out[q, v, (pair, c)] -> [b, i, c] ; normalize by denominator col
    outs = []
    for c in range(NCORES):
        o = res.results[c]["out"].astype(np.float32)         # [NOCT, 128, 528]
        o = (o.reshape(NOCT, 2, N, 4, HHW)                   # [q, u, i, pair, c]
             .transpose(0, 3, 1, 2, 4)                       # [q, pair, u, i, c]
             .reshape(BPC, N, HHW))
        outs.append(o[:, :, 0:D] / o[:, :, D:D + 1])
    out = np.concatenate(outs, axis=0)
    return out.reshape(B, N, D).astype(np.float32), res


def kernel(hidden, adj, a):
    out, _ = run_device(hidden, adj, a)
    return out
